# revision 1
# baseline (speedup 1.0000x reference)
"""Pairwise-interaction kernel for Trainium2 (raw Bass), 8-core SPMD.

Computes out[b, p, :] = x[b, i(p), :] * x[b, j(p), :] for all pairs
(i < j) of the F=26 feature rows, p ordered row-major (i outer, j inner).

Sharding: data-parallel over the batch dim (16384 -> 8 x 2048), no
cross-core communication. Per core: 16 tiles of 128 samples on SBUF
partitions. For each tile the "i" row is broadcast (stride-0 AP) against
the contiguous tail x[:, i+1:] with one fp32 tensor_tensor multiply per
i (25 per tile), writing a compact [128, 10400] output tile stored with
a single 5.3MB DMA.

Raw-Bass sync scheme (every instruction carries at most ONE semaphore
wait — the ISA allows exactly one wait slot per instruction):
  sem_ld  (+16 per load DMA, scalar/ACT HWDGE ring)
  sem_st  (+16 per store DMA, sync/SP HWDGE ring)
  sem_tt  (+1 by the last TT of each tile, vector engine)
  loads   wait sem_tt >= t-XB+1   (previous tenant's TTs have read the slot)
  vector  waits sem_ld >= 16(t+1) and sem_st >= 16(t-YB+1) as standalone
          wait ops, then runs the 25 TTs wait-free
  stores  wait sem_tt >= t+1      (this tile's TTs are done)
"""

import numpy as np

import concourse.bass as bass
from concourse import mybir
from concourse.bass_utils import run_bass_kernel_spmd

B, F, D = 16384, 26, 32
NCORES = 8
BC = B // NCORES           # 2048 samples per core
P = 128                    # SBUF partitions per tile
NT = BC // P               # 16 tiles per core
FD = F * D                 # 832
NPAIR = F * (F - 1) // 2   # 325
OD = NPAIR * D             # 10400

XB = 3                     # input tile buffers
YB = 2                     # output tile buffers

# Chunked stores: split each tile's 325 pair-rows into 4 chunks of
# consecutive i-blocks so the store of a chunk overlaps the compute of the
# next one (shrinks pipeline fill/drain). Chunks are (i_lo, i_hi) ranges.
CHUNKS = [(0, 3), (3, 7), (7, 12), (12, 25)]
NCH = len(CHUNKS)


def _chunk_pair_off(i_lo):
    # first output pair index for block i = i_lo
    return sum(F - 1 - i for i in range(i_lo))

F32 = mybir.dt.float32

_nc_cache = None


def _build_nc():
    nc = bass.Bass()
    x = nc.declare_dram_parameter("x", [BC, FD], F32, isOutput=False)
    y = nc.declare_dram_parameter("y", [BC, OD], F32, isOutput=True)
    xv = x[:].rearrange("(n p) m -> n p m", p=P)
    yv = y[:].rearrange("(n p) m -> n p m", p=P)

    with (
        nc.sbuf_tensor([P, XB * FD], F32) as xbuf,
        nc.sbuf_tensor([P, YB * OD], F32) as ybuf,
        nc.semaphore("sem_ld") as sem_ld,
        nc.semaphore("sem_st") as sem_st,
        nc.semaphore("sem_tt") as sem_tt,
        nc.Block() as blk,
    ):
        xts = [xbuf[:, b * FD : (b + 1) * FD] for b in range(XB)]
        yts = [ybuf[:, b * OD : (b + 1) * OD] for b in range(YB)]

        @blk.scalar
        def _(scalar):
            for t in range(NT):
                ld = scalar.dma_start(xts[t % XB], xv[t])
                if t >= XB:
                    # Slot free once the previous tenant's TTs have read it
                    # (sem_tt counts chunk completions, NCH per tile).
                    ld._wait_ge(sem_tt, NCH * (t - XB + 1))
                ld.then_inc(sem_ld, 16)

        @blk.sync
        def _(sync):
            for t in range(NT):
                for c, (i_lo, i_hi) in enumerate(CHUNKS):
                    p_lo = _chunk_pair_off(i_lo)
                    p_hi = _chunk_pair_off(i_hi)
                    st = sync.dma_start(
                        yv[t][:, p_lo * D : p_hi * D],
                        yts[t % YB][:, p_lo * D : p_hi * D],
                    )
                    st._wait_ge(sem_tt, NCH * t + c + 1)
                    st.then_inc(sem_st, 16)

        @blk.vector
        def _(v):
            for t in range(NT):
                xt = xts[t % XB]
                yt = yts[t % YB]
                v.wait_ge(sem_ld, 16 * (t + 1))
                for c, (i_lo, i_hi) in enumerate(CHUNKS):
                    if t >= YB:
                        # chunk c of tile t-YB has been stored
                        v.wait_ge(sem_st, 16 * (NCH * (t - YB) + c + 1))
                    off = _chunk_pair_off(i_lo)
                    for i in range(i_lo, i_hi):
                        nrep = F - 1 - i
                        in0 = (
                            xt[:, i * D : (i + 1) * D]
                            .unsqueeze(1)
                            .broadcast_to([P, nrep, D])
                        )
                        in1 = xt[:, (i + 1) * D : FD].rearrange(
                            "p (r d) -> p r d", d=D
                        )
                        outap = yt[:, off * D : (off + nrep) * D].rearrange(
                            "p (r d) -> p r d", d=D
                        )
                        tt = nc.vector.tensor_mul(outap, in0, in1)
                        off += nrep
                    tt.then_inc(sem_tt, 1)

    return nc


def kernel(inputs: np.ndarray) -> np.ndarray:
    global _nc_cache
    if _nc_cache is None:
        _nc_cache = _build_nc()
    nc = _nc_cache

    x = np.ascontiguousarray(np.asarray(inputs, dtype=np.float32)).reshape(B, FD)
    shards = x.reshape(NCORES, BC, FD)
    in_maps = [{"x": shards[c]} for c in range(NCORES)]
    res = run_bass_kernel_spmd(nc, in_maps, list(range(NCORES)))
    out = np.concatenate(
        [res.results[c]["y"].reshape(BC, NPAIR, D) for c in range(NCORES)], axis=0
    )
    return out



# revision 2
# speedup vs baseline: 1.9545x; 1.9545x over previous
"""Pairwise-interaction kernel for Trainium2 (raw Bass), 8-core SPMD.

Computes out[b, p, :] = x[b, i(p), :] * x[b, j(p), :] for all pairs
(i < j) of the F=26 feature rows, p ordered row-major (i outer, j inner).

Sharding: data-parallel over the batch dim (16384 -> 8 x 2048), no
cross-core communication. Per core: 16 tiles of 128 samples on SBUF
partitions. For each tile the "i" row is broadcast (stride-0 AP) against
the contiguous tail x[:, i+1:] with one tensor_tensor multiply per
i (25 per tile), writing a compact [128, 10400] output tile stored in
4 chunked DMAs.

The kernel is HBM-bound (output is 12.5x the input), so all device
traffic is bf16: the host converts the f32 input to bf16 (RN), the DVE
multiplies bf16*bf16 -> bf16 (2x perf mode), and the host upcasts the
bf16 output back to f32. Worst-case per-element relative error is
~3*2^-8 = 1.2% (two input roundings + one output rounding), inside the
2e-2 gate. bf16 halves HBM traffic vs f32: 92 MB -> 46 MB per core.

Raw-Bass sync scheme (every instruction carries at most ONE semaphore
wait — the ISA allows exactly one wait slot per instruction):
  sem_ld  (+16 per load DMA, scalar/ACT HWDGE ring)
  sem_st  (+16 per store DMA, sync/SP HWDGE ring)
  sem_tt  (+1 by the last TT of each chunk, vector engine)
  loads   wait sem_tt >= NCH*(t-XB+1)  (previous tenant's TTs done)
  vector  waits sem_ld >= 16(t+1) and sem_st >= 16*... as standalone
          wait ops, then runs the TTs wait-free
  stores  wait sem_tt >= NCH*t + c + 1 (this chunk's TTs are done)
"""

import numpy as np
import ml_dtypes

import concourse.bass as bass
from concourse import mybir
from concourse.bass_utils import run_bass_kernel_spmd

B, F, D = 16384, 26, 32
NCORES = 8
BC = B // NCORES           # 2048 samples per core
P = 128                    # SBUF partitions per tile
NT = BC // P               # 16 tiles per core
FD = F * D                 # 832
NPAIR = F * (F - 1) // 2   # 325
OD = NPAIR * D             # 10400

XB = 3                     # input tile buffers
YB = 2                     # output tile buffers

# Chunked stores: split each tile's 325 pair-rows into 4 chunks of
# consecutive i-blocks so the store of a chunk overlaps the compute of the
# next one (shrinks pipeline fill/drain). Chunks are (i_lo, i_hi) ranges.
CHUNKS = [(0, 3), (3, 7), (7, 12), (12, 25)]
NCH = len(CHUNKS)


def _chunk_pair_off(i_lo):
    # first output pair index for block i = i_lo
    return sum(F - 1 - i for i in range(i_lo))


DT = mybir.dt.bfloat16
NPDT = ml_dtypes.bfloat16

_nc_cache = None


def _build_nc():
    nc = bass.Bass()
    x = nc.declare_dram_parameter("x", [BC, FD], DT, isOutput=False)
    y = nc.declare_dram_parameter("y", [BC, OD], DT, isOutput=True)
    xv = x[:].rearrange("(n p) m -> n p m", p=P)
    yv = y[:].rearrange("(n p) m -> n p m", p=P)

    with (
        nc.sbuf_tensor([P, XB * FD], DT) as xbuf,
        nc.sbuf_tensor([P, YB * OD], DT) as ybuf,
        nc.semaphore("sem_ld") as sem_ld,
        nc.semaphore("sem_st") as sem_st,
        nc.semaphore("sem_tt") as sem_tt,
        nc.Block() as blk,
    ):
        xts = [xbuf[:, b * FD : (b + 1) * FD] for b in range(XB)]
        yts = [ybuf[:, b * OD : (b + 1) * OD] for b in range(YB)]

        @blk.scalar
        def _(scalar):
            for t in range(NT):
                ld = scalar.dma_start(xts[t % XB], xv[t])
                if t >= XB:
                    # Slot free once the previous tenant's TTs have read it
                    # (sem_tt counts chunk completions, NCH per tile).
                    ld._wait_ge(sem_tt, NCH * (t - XB + 1))
                ld.then_inc(sem_ld, 16)

        @blk.sync
        def _(sync):
            for t in range(NT):
                for c, (i_lo, i_hi) in enumerate(CHUNKS):
                    p_lo = _chunk_pair_off(i_lo)
                    p_hi = _chunk_pair_off(i_hi)
                    st = sync.dma_start(
                        yv[t][:, p_lo * D : p_hi * D],
                        yts[t % YB][:, p_lo * D : p_hi * D],
                    )
                    st._wait_ge(sem_tt, NCH * t + c + 1)
                    st.then_inc(sem_st, 16)

        @blk.vector
        def _(v):
            for t in range(NT):
                xt = xts[t % XB]
                yt = yts[t % YB]
                v.wait_ge(sem_ld, 16 * (t + 1))
                for c, (i_lo, i_hi) in enumerate(CHUNKS):
                    if t >= YB:
                        # chunk c of tile t-YB has been stored
                        v.wait_ge(sem_st, 16 * (NCH * (t - YB) + c + 1))
                    off = _chunk_pair_off(i_lo)
                    for i in range(i_lo, i_hi):
                        nrep = F - 1 - i
                        in0 = (
                            xt[:, i * D : (i + 1) * D]
                            .unsqueeze(1)
                            .broadcast_to([P, nrep, D])
                        )
                        in1 = xt[:, (i + 1) * D : FD].rearrange(
                            "p (r d) -> p r d", d=D
                        )
                        outap = yt[:, off * D : (off + nrep) * D].rearrange(
                            "p (r d) -> p r d", d=D
                        )
                        tt = nc.vector.tensor_mul(outap, in0, in1)
                        off += nrep
                    tt.then_inc(sem_tt, 1)

    return nc


def make_in_maps(inputs):
    """f32 [B, F, D] -> per-core bf16 shard maps (host-side RN rounding)."""
    x = (
        np.ascontiguousarray(np.asarray(inputs, dtype=np.float32))
        .reshape(B, FD)
        .astype(NPDT)
    )
    shards = x.reshape(NCORES, BC, FD)
    return [{"x": shards[c]} for c in range(NCORES)]


def kernel(inputs: np.ndarray) -> np.ndarray:
    global _nc_cache
    if _nc_cache is None:
        _nc_cache = _build_nc()
    nc = _nc_cache

    in_maps = make_in_maps(inputs)
    res = run_bass_kernel_spmd(nc, in_maps, list(range(NCORES)))
    out = np.empty((B, NPAIR, D), dtype=np.float32)
    for c in range(NCORES):
        out[c * BC : (c + 1) * BC] = (
            res.results[c]["y"].reshape(BC, NPAIR, D).astype(np.float32)
        )
    return out
